# revision 65
# baseline (speedup 1.0000x reference)
"""Multi-head self-attention (RoPE, causal) on 8 Trainium2 NeuronCores.

Sharding: core c -> (batch = c//2, head-group = c%2 of 8 heads).
Column-parallel wq/wk/wv, row-parallel wo. Each core emits a partial
out^T [f, s]; the host sums the two partials per batch and transposes.

Layouts (all chosen so no on-device transposes are needed):
  XT  [d, s]   (x transposed on host, bf16)
  Q^T/K^T [e, s] per head from matmul(lhsT=wT[d,e], rhs=XT[d,s])
  V   [s, e]   from matmul(lhsT=XT[d,s], rhs=wvT[d,e])
  S^T [j, i] = matmul(lhsT=K^T[e,j], rhs=Q^T[e,i])
  ctx^T [e, i] = matmul(lhsT=V[j,e], rhs=expS^T[j,i])
  out^T [f, s] = matmul(lhsT=woT[d,f], rhs=ctx^T[d,s])

All DRAM inputs/outputs are pre-tiled on the host into the exact
[128, N] blocks each DMA moves, so every DMA is a dense contiguous
copy. All matmul operands are bf16 (PSUM accumulation stays fp32);
softmax statistics and RoPE arithmetic stay fp32.

RoPE: head dims de-interleaved on host (even dims -> partitions 0..63,
odd -> 64..127 of each head's Q^T/K^T) by permuting wq/wk rows. Then
rot(x) = x*cc + swap_halves(x)*ss where the half-swap is two SBUF->SBUF
partition-shifted DMAs (no tensor-engine work) and ss carries the
rotation signs (-sin on partitions 0..63, +sin on 64..127). The
1/sqrt(dk) scale is applied via the Exp activation's scale field.

Softmax: no max-subtraction (scores are O(1)-scaled; fp32 exp is safe).
Causal masking by block-skipping + one 128x128 triangular mask on
diagonal blocks. Row sums: exp tiles are pair-summed on DVE into an
fp32r accumulator per i-block, then a single ones-matmul (fp32r runs at
bf16 speed for 512-wide moving tiles) gives the per-column sums
broadcast across partitions; normalization multiplies ctx^T by a fast
DVE reciprocal of that tile.

Scheduling (from perfetto analysis; 620us -> ~606us):
 - ~36 junk matmuls at t=0 warm the PE HAM clock gate (1.2->2.4 GHz)
   during the otherwise-idle startup DMA window; more junk matmuls are
   interleaved into the DMA-paced startup chains (head 0 chunk 0 and
   the first V tiles) so the activity window never re-throttles.
 - The SP DMA queue is kept just-in-time ordered: the bulk xt/cc/ss
   chunk loads for chunks 1..3 are emitted inside head 0's i-block
   bodies so the small latency-critical rope half-swap DMAs are never
   head-of-line blocked behind megabytes of prefetch (the old ordering
   cost ~7us of PE stall at head 0/1).
 - wqk for head 1 is enqueued right after wv group 0, long before the
   tail of the xt stream.
 - V lives in one half-sized buffer: group 0 (heads 0-3) until h3's
   last AV matmul, then the slot is ring-reused for group 1 at h4.
 - wo is prefetched during head 5 into the SBUF slot wv vacates (pool
   tag reuse), via a single scalar-engine (ACT hwdge queue) DMA so it
   does not contend with the SP queue.
 - Each head's i-block-0 diagonal scores+exps are hoisted right after
   projection chunk 0, giving the ACT exps a long head start.
 - Row sums use a bf16 ones-matmul (fp32r moving operands run ~2x
   slower on the PE); the 4 diag adds stay fp32.
 - The output projection is interleaved with head 7's attention: the
   7-head partial accumulations (ctx staged in DRAM, read back into
   the slot xt vacates; sb4=0 gets a dedicated early buffer) fill the
   PE while head 7's exps drain. Out chains rotate over all 8 PSUM
   banks and their evacuation copies are split ACT/DVE so no engine's
   FIFO backlog stalls chain starts; the final tile runs as two halves
   so the last copy+DMA overlap.
 - Output partials are written bf16 (summed fp32 on host).
"""

import numpy as np
import ml_dtypes

import concourse.bass as bass
import concourse.tile as tile
import concourse.mybir as mybir
from concourse import bacc, bass_utils

F32 = mybir.dt.float32
F32R = mybir.dt.float32r
BF16 = mybir.dt.bfloat16

B = 4
S = 2048
D = 2048
NH = 16
DK = 128
NCORES = 8
HPC = 8            # heads per core
DLOC = HPC * DK    # 1024, local model dims per core
ST = S // 128      # 16 sequence 128-tiles
DT = D // 128      # 16 model-dim 128-tiles
NDT = DLOC // 128  # 8 local model-dim 128-tiles
IB = S // 512      # 4 i-blocks of 512
ROPE_THETA = 10000.0
SCALE = float(1.0 / np.sqrt(DK))

_cache = {}


def build_program():
    if "nc" in _cache:
        return _cache["nc"]

    nc = bacc.Bacc("TRN2", target_bir_lowering=False, debug=False,
                   num_devices=NCORES)

    xt = nc.dram_tensor("xt", [DT, 4, 128, 512], BF16, kind="ExternalInput").ap()
    wq = nc.dram_tensor("wq", [HPC, DT, 128, DK], BF16, kind="ExternalInput").ap()
    wk = nc.dram_tensor("wk", [HPC, DT, 128, DK], BF16, kind="ExternalInput").ap()
    wv = nc.dram_tensor("wv", [2, DT, 128, 512], BF16, kind="ExternalInput").ap()
    wo = nc.dram_tensor("wo", [DT, 128, DLOC], BF16, kind="ExternalInput").ap()
    cct = nc.dram_tensor("cct", [128, S], BF16, kind="ExternalInput").ap()
    sst = nc.dram_tensor("sst", [128, S], BF16, kind="ExternalInput").ap()
    tri = nc.dram_tensor("tri", [128, 128], BF16, kind="ExternalInput").ap()
    out = nc.dram_tensor("out", [DT, IB, 128, 512], BF16,
                         kind="ExternalOutput").ap()

    with tile.TileContext(nc) as tc:
        with (
            tc.tile_pool(name="dram", bufs=1, space="DRAM") as dram_pool,
            tc.tile_pool(name="ctx7", bufs=2) as ctx7_pool,
        ):
            ctx_dram = dram_pool.tile([HPC, IB, 128, 512], BF16)
            _body(nc, tc, xt, wq, wk, wv, wo, cct, sst, tri, ctx_dram, out,
                  ctx7_pool)

    nc.compile()
    _cache["nc"] = nc
    return nc


def _body(nc, tc, xt, wq, wk, wv, wo, cct, sst, tri, ctx_dram, out,
          ctx7_pool):
    with (
        tc.tile_pool(name="xt", bufs=1) as xt_pool,
        tc.tile_pool(name="vsb", bufs=1) as v_pool,
        tc.tile_pool(name="tabs", bufs=1) as tab_pool,
        tc.tile_pool(name="wqk", bufs=2) as wqk_pool,
        tc.tile_pool(name="qkraw", bufs=2) as raw_pool,
        tc.tile_pool(name="rqk", bufs=2) as rqk_pool,
        tc.tile_pool(name="outsb", bufs=2) as out_pool,
        tc.tile_pool(name="cin0", bufs=1) as cin0_pool,
        tc.tile_pool(name="qk_ps", bufs=2, space="PSUM") as qk_ps_pool,
        tc.tile_pool(name="s_ps", bufs=3, space="PSUM") as s_ps_pool,
        tc.tile_pool(name="rs_ps", bufs=1, space="PSUM") as rs_ps_pool,
    ):
        # ---- PE warmup: junk matmuls during the startup DMA window so the
        # HAM clock gate reaches 2.4 GHz before the first real matmul ----
        ones_f32 = tab_pool.tile([128, 128], F32, tag="ones_f32")
        nc.gpsimd.memset(ones_f32[:], 1.0)
        warm_bf = tab_pool.tile([128, 128], BF16, tag="warm_bf")
        nc.vector.tensor_copy(warm_bf[:], ones_f32[:])
        # junk matmuls target the rowsum PSUM bank: its first real use is
        # ~30us in, so startup junk interleaved into DMA-paced chains can
        # keep writing it without disturbing any live accumulation ring
        warm_ps = rs_ps_pool.tile([128, 512], F32, tag="rs")
        for _ in range(36):
            nc.tensor.matmul(warm_ps[:, 0:128], warm_bf[:], warm_bf[:],
                             start=True, stop=True)

        def junk_mm():
            # keep the PE's HAM activity window busy during DMA-paced
            # startup stalls so real matmuls run at 2.4 GHz, not 1.2
            nc.tensor.matmul(warm_ps[:, 0:128], warm_bf[:], warm_bf[:],
                             start=True, stop=True, skip_group_check=True)

        # ---- resident loads (dense contiguous DMAs) ----
        def load_wqk(h):
            wq_sb = wqk_pool.tile([128, DT, DK], BF16, tag="wq")
            wk_sb = wqk_pool.tile([128, DT, DK], BF16, tag="wk")
            nc.sync.dma_start(wk_sb[:], wk[h].rearrange("d p k -> p d k",
                                                       p=128))
            nc.sync.dma_start(wq_sb[:], wq[h].rearrange("d p k -> p d k",
                                                       p=128))
            return wq_sb, wk_sb

        xt_sb = xt_pool.tile([128, DT, S], BF16, tag="xt")
        wv_sb = tab_pool.tile([128, DT, DLOC], BF16, tag="wv")
        cc_sb = tab_pool.tile([128, S], BF16, tag="cct")
        ss_sb = tab_pool.tile([128, S], BF16, tag="sst")
        tri_sb = tab_pool.tile([128, 128], BF16, tag="tri")
        nc.sync.dma_start(tri_sb[:], tri)

        # startup-critical order: exactly the just-in-time consumption order
        # of head 0's first accumulation chain (wk/xt k-tile groups
        # interleaved), then the rest of the ib0 working set; chunk 1-3
        # loads are woven into head 0's i-block bodies below so the rope
        # half-swap DMAs are never far back in the SP queue
        wq0_sb = wqk_pool.tile([128, DT, DK], BF16, tag="wq")
        wk0_sb = wqk_pool.tile([128, DT, DK], BF16, tag="wk")
        wqk0 = (wq0_sb, wk0_sb)
        for q in range(0, DT, 2):
            nc.sync.dma_start(
                wk0_sb[:, q:q + 2, :],
                wk[0, q:q + 2].rearrange("d p k -> p d k", p=128))
            nc.sync.dma_start(
                xt_sb[:, q:q + 2, 0:512],
                xt[q:q + 2, 0].rearrange("d p c -> p d c", p=128))
        nc.sync.dma_start(wq0_sb[:], wq[0].rearrange("d p k -> p d k", p=128))
        nc.sync.dma_start(cc_sb[:, 0:512], cct[:, 0:512])
        nc.sync.dma_start(ss_sb[:, 0:512], sst[:, 0:512])

        def load_chunk(ch):
            for q in range(0, DT, 8):
                nc.sync.dma_start(
                    xt_sb[:, q:q + 8, ch * 512:(ch + 1) * 512],
                    xt[q:q + 8, ch].rearrange("d p c -> p d c", p=128))
            o = ch * 512
            nc.sync.dma_start(cc_sb[:, o:o + 512], cct[:, o:o + 512])
            nc.sync.dma_start(ss_sb[:, o:o + 512], sst[:, o:o + 512])

        def proj_chunk(w_sb, r_t, ch, junk=False):
            o = ch * 512
            ps = qk_ps_pool.tile([128, 512], F32, tag="qk_ps")
            for dt in range(DT):
                nc.tensor.matmul(
                    ps[:],
                    w_sb[:, dt, :],
                    xt_sb[:, dt, o:o + 512],
                    start=(dt == 0), stop=(dt == DT - 1),
                    skip_group_check=junk,
                )
                if junk and dt < DT - 1:
                    junk_mm()
            raw = raw_pool.tile([128, 512], BF16, tag="qkraw")
            nc.scalar.copy(raw[:], ps[:])
            # rope half-swap via two partition-shifted SBUF->SBUF DMAs;
            # the rotation signs live in the ss table
            swp = raw_pool.tile([128, 512], BF16, tag="swp")
            nc.sync.dma_start(swp[0:64, :], raw[64:128, :])
            nc.sync.dma_start(swp[64:128, :], raw[0:64, :])
            nc.vector.tensor_mul(swp[:], swp[:], ss_sb[:, o:o + 512])
            t3 = raw_pool.tile([128, 512], BF16, tag="t3")
            nc.vector.tensor_mul(t3[:], raw[:], cc_sb[:, o:o + 512])
            nc.vector.tensor_add(r_t[:, o:o + 512], swp[:], t3[:])

        def emit_diag(rq, rk, ib):
            # the 4 diagonal score tiles of i-block ib (+ exp + tri mask);
            # hoisted right after projection chunk ib's rope so the ACT
            # exps get a long head start on the AV matmuls that use them
            i0 = ib * 512
            diag_es = []
            for r in range(4):
                jt = 4 * ib + r
                lo = 128 * r
                s_ps = s_ps_pool.tile([128, 512], F32, tag="s_ps")
                nc.tensor.matmul(
                    s_ps[:, lo:512],
                    rk[:, jt * 128:(jt + 1) * 128],
                    rq[:, i0 + lo:i0 + 512],
                    start=True, stop=True,
                )
                es = exp_pool.tile([128, 512], BF16, tag="exps")
                nc.scalar.activation(es[:, lo:512], s_ps[:, lo:512],
                                     mybir.ActivationFunctionType.Exp,
                                     scale=SCALE)
                nc.vector.tensor_mul(es[:, lo:lo + 128],
                                     es[:, lo:lo + 128], tri_sb[:])
                diag_es.append((jt, lo, es))
            return diag_es

        def proj_rope(wq_sb, wk_sb):
            rq = rqk_pool.tile([128, S], BF16, tag="rq")
            rk = rqk_pool.tile([128, S], BF16, tag="rk")
            proj_chunk(wk_sb, rk, 0)
            proj_chunk(wq_sb, rq, 0)
            pre_diag = emit_diag(rq, rk, 0)
            for ch in range(1, 4):
                proj_chunk(wk_sb, rk, ch)
                proj_chunk(wq_sb, rq, ch)
            return rq, rk, pre_diag

        # head 0's projection is emitted per-chunk, interleaved with its
        # attention i-blocks (chunk ib is exactly what i-block ib consumes),
        # so DMA-paced chunks don't head-of-line-block ready attention work.
        # Chunk 0 is emitted BEFORE the wv/wqk1 bulk loads so its rope
        # half-swap DMAs sit at ~3.5MB in the SP queue, not ~6.5MB — the
        # hoisted ib0 diagonal scores need the rotated rq0/rk0 early.
        rq0 = rqk_pool.tile([128, S], BF16, tag="rq")
        rk0 = rqk_pool.tile([128, S], BF16, tag="rk")
        proj_chunk(wqk0[1], rk0, 0, junk=True)
        proj_chunk(wqk0[0], rq0, 0, junk=True)
        rqk0 = (rq0, rk0)
        for q in range(0, DT, 4):
            nc.sync.dma_start(wv_sb[:, q:q + 4, 0:512],
                             wv[0, q:q + 4].rearrange("d p c -> p d c",
                                                      p=128))
        wqk_next = load_wqk(1)

        # ---- V = x @ wv.T (emitted interleaved with head-0 attention) ----
        # One half-sized V buffer: group 0 (heads 0-3) lives until h3's
        # last AV matmul, then the slot is ring-reused for group 1 at h4.

        def emit_v(v_sb, st, g, junk=False):
            v_ps = qk_ps_pool.tile([128, 512], F32, tag="qk_ps")
            for dt in range(DT):
                nc.tensor.matmul(
                    v_ps[:],
                    xt_sb[:, dt, st * 128:(st + 1) * 128],
                    wv_sb[:, dt, g * 512:(g + 1) * 512],
                    start=(dt == 0), stop=(dt == DT - 1),
                    skip_group_check=junk,
                )
                if junk and dt < DT - 1:
                    junk_mm()
            nc.vector.tensor_copy(v_sb[:, st, :], v_ps[:])

        # ---- per-head attention (+ next head's projection interleaved) ----
        with (
            tc.tile_pool(name="exps", bufs=6) as exp_pool,
            tc.tile_pool(name="pair", bufs=2) as pair_pool,
            tc.tile_pool(name="acc", bufs=2) as acc_pool,
            tc.tile_pool(name="small", bufs=2) as small_pool,
            tc.tile_pool(name="ctxsb", bufs=2) as ctx_sb_pool,
            tc.tile_pool(name="ctx_ps", bufs=2, space="PSUM") as ctx_ps_pool,
        ):
            def emit_out_chain(ft, ib, cin_src, ctx_sb, wo_sb):
                half = ft // 8
                fo = (ft % 8) * 128
                # rotate chains over all 8 PSUM banks; the ctx_ps/rs_ps
                # slots are taken mid-block, after this i-block's DVE tail
                # (ctx mul, recip) has certainly freed them — so no chain
                # start ever waits on a burst-delayed copy or DVE op
                if ft in (8, 9):
                    ps = ctx_ps_pool.tile([128, 512], F32, tag="ctx_ps")
                elif ft == 10:
                    ps = rs_ps_pool.tile([128, 512], F32, tag="rs")
                elif ft % 2 == 1:
                    ps = qk_ps_pool.tile([128, 512], F32, tag="qk_ps")
                else:
                    ps = s_ps_pool.tile([128, 512], F32, tag="s_ps")
                # the very last chain runs as two half-width chains in
                # DIFFERENT PSUM banks (PE-write + engine-read of the same
                # bank serialize) with copies on different engines, so the
                # final copy+DMA overlap the second half's matmuls
                if ib == IB - 1 and ft == DT - 1:
                    ps2 = s_ps_pool.tile([128, 512], F32, tag="s_ps")
                    halves = ((0, 256, ps, nc.scalar.copy),
                              (256, 512, ps2, nc.vector.tensor_copy))
                else:
                    copier = (nc.scalar.copy if ft % 3 == 1
                              else nc.vector.tensor_copy)
                    halves = ((0, 512, ps, copier),)
                osb = out_pool.tile([128, 512], BF16, tag="osb")
                for (c0, c1, psx, copy_fn) in halves:
                    for dt in range(NDT - 1):
                        nc.tensor.matmul(
                            psx[:, c0:c1],
                            wo_sb[:, 2 * dt + half, fo:fo + 128],
                            cin_src[:, dt, c0:c1],
                            start=(dt == 0), stop=False,
                        )
                    nc.tensor.matmul(
                        psx[:, c0:c1],
                        wo_sb[:, 2 * (NDT - 1) + half, fo:fo + 128],
                        ctx_sb[:, c0:c1],
                        start=False, stop=True,
                    )
                    # evacuation copies mostly on ACT (only copies sit in
                    # its FIFO during an out block); DVE takes a share
                    copy_fn(osb[:, c0:c1], psx[:, c0:c1])
                    nc.sync.dma_start(out[ft, ib][:, c0:c1], osb[:, c0:c1])

            ctx7 = []
            wo_sb = None
            cin = None
            pre_diag0 = emit_diag(rq0, rk0, 0)
            v_sb = v_pool.tile([128, ST, 512], BF16, tag="v")
            for h in range(HPC):
                if h == 0:
                    rq, rk = rqk0
                    pre_diag = pre_diag0
                else:
                    w_cur, wqk_next = wqk_next, None
                    rq, rk, pre_diag = proj_rope(*w_cur)
                if 0 < h < HPC - 1:
                    wqk_next = load_wqk(h + 1)
                if h == 5:
                    # prefetch wo into the SBUF slot wv vacates after the
                    # group-1 V matmuls (pool tag ring); single ACT-queue
                    # DMA so the SP queue's JIT order is untouched
                    wo_sb = tab_pool.tile([128, DT, DLOC], BF16, tag="wv")
                    nc.scalar.dma_start(
                        wo_sb[:], wo.rearrange("j p c -> p j c", p=128))
                if h == HPC - 1:
                    # sb4=0's ctx readback has a dedicated buffer loaded
                    # before head 7's projection, so the first out chains
                    # don't wait for the xt slot to free + DMA latency;
                    # sb4 1-3 reuse the slot xt vacates (pool tag ring)
                    cin0 = cin0_pool.tile([128, NDT - 1, 512], BF16)
                    nc.sync.dma_start(
                        cin0[:],
                        ctx_dram[0:NDT - 1, 0].rearrange(
                            "h p c -> p h c", p=128))
                    cin = xt_pool.tile([128, IB, DT, 512], BF16, tag="xt")
                    for sb4 in range(1, IB):
                        nc.sync.dma_start(
                            cin[:, sb4, 0:NDT - 1, :],
                            ctx_dram[0:NDT - 1, sb4].rearrange(
                                "h p c -> p h c", p=128))

                for ib in range(IB):
                    if h == 0:
                        if ib > 0:
                            proj_chunk(wqk0[1], rk, ib)
                            proj_chunk(wqk0[0], rq, ib)
                        # V tiles this i-block needs (g=0), just in time
                        for st in range(4 * ib, 4 * ib + 4):
                            emit_v(v_sb, st, 0, junk=(ib == 0))
                        # chunk ib+1 bulk loads enter the SP queue here,
                        # behind this i-block's rope swaps
                        if ib < 3:
                            load_chunk(ib + 1)
                        else:
                            nc.sync.dma_start(
                                wv_sb[:, :, 512:1024],
                                wv[1].rearrange("d p c -> p d c", p=128))
                    elif h == 4 and ib == 0:
                        # group-1 V (heads 4-7) into the slot group 0
                        # vacates after h3's last AV matmul (pool tag ring)
                        v_sb = v_pool.tile([128, ST, 512], BF16, tag="v")
                        for st in range(ST):
                            emit_v(v_sb, st, 1)
                    i0 = ib * 512
                    ctx_ps = ctx_ps_pool.tile([128, 512], F32, tag="ctx_ps")
                    acc = acc_pool.tile([128, 512], F32, tag="acc")
                    es_prev = None
                    acc_started = False
                    for jt in range(4 * ib):
                        s_ps = s_ps_pool.tile([128, 512], F32, tag="s_ps")
                        nc.tensor.matmul(
                            s_ps[:],
                            rk[:, jt * 128:(jt + 1) * 128],
                            rq[:, i0:i0 + 512],
                            start=True, stop=True,
                        )
                        es = exp_pool.tile([128, 512], BF16, tag="exps")
                        nc.scalar.activation(es[:], s_ps[:],
                                             mybir.ActivationFunctionType.Exp,
                                             scale=SCALE)
                        # row sums: DVE pair-adds into an fp32 accumulator
                        if jt % 2 == 0:
                            es_prev = es
                        elif not acc_started:
                            nc.vector.tensor_add(acc[:], es_prev[:], es[:])
                            acc_started = True
                        else:
                            pair = pair_pool.tile([128, 512], BF16,
                                                  tag="pair")
                            nc.vector.tensor_add(pair[:], es_prev[:], es[:])
                            nc.vector.tensor_add(acc[:], acc[:], pair[:])
                        nc.tensor.matmul(
                            ctx_ps[:],
                            v_sb[:, jt, (h % 4) * DK:(h % 4 + 1) * DK],
                            es[:],
                            start=(jt == 0), stop=False,
                            skip_group_check=True,
                        )
                    # diagonal scores were emitted ahead (ib 0 during the
                    # projection; later i-blocks here) so the exps pipeline
                    diag_es = pre_diag if ib == 0 else emit_diag(rq, rk, ib)
                    for r, (jt, lo, es) in enumerate(diag_es):
                        if r == 0 and not acc_started:
                            nc.vector.tensor_copy(acc[:], es[:])
                            acc_started = True
                        else:
                            nc.vector.tensor_add(acc[:, lo:512],
                                                 acc[:, lo:512],
                                                 es[:, lo:512])
                        nc.tensor.matmul(
                            ctx_ps[:, lo:512],
                            v_sb[:, jt, (h % 4) * DK:(h % 4 + 1) * DK],
                            es[:, lo:512],
                            start=(jt == 0), stop=(r == 3),
                            skip_group_check=True,
                        )
                    # rowsum via bf16 ones-matmul (fp32r moving operand runs
                    # ~2x slower); one bf16 rounding per partition partial
                    # averages out to ~0.04% error on the 128-term sum
                    accb = pair_pool.tile([128, 512], BF16, tag="pair")
                    nc.vector.tensor_copy(accb[:], acc[:])
                    rs_ps = rs_ps_pool.tile([128, 512], F32, tag="rs")
                    nc.tensor.matmul(
                        rs_ps[:],
                        warm_bf[:],
                        accb[:],
                        start=True, stop=True,
                    )
                    recip = small_pool.tile([128, 512], F32, tag="recip")
                    nc.vector.reciprocal_approx_fast(recip[:], rs_ps[:])
                    if h == HPC - 1:
                        ctx_sb = ctx7_pool.tile([128, 512], BF16, tag="c7")
                        ctx7.append(ctx_sb)
                    else:
                        ctx_sb = ctx_sb_pool.tile([128, 512], BF16,
                                                  tag="ctx_sb")
                    nc.vector.tensor_mul(ctx_sb[:], ctx_ps[:], recip[:])
                    if h != HPC - 1:
                        nc.sync.dma_start(ctx_dram[h, ib], ctx_sb[:])
                    else:
                        # output projection for this i-block: 7 partials
                        # from cin, then head 7's SBUF-resident ctx last
                        cin_src = cin0 if ib == 0 else cin[:, ib]
                        for ft in range(DT):
                            emit_out_chain(ft, ib, cin_src, ctx_sb, wo_sb)


def _tile2(a, p, q):
    """[R, C] -> [R//p, C//q, p, q] contiguous blocks."""
    R, C = a.shape
    return np.ascontiguousarray(
        a.reshape(R // p, p, C // q, q).transpose(0, 2, 1, 3))


def prepare_in_maps(x, wq, wk, wv, wo):
    """Build the 8 per-core input maps (host-side sharding + tables)."""
    x = np.asarray(x, dtype=np.float32)
    wq = np.asarray(wq, dtype=np.float32)
    wk = np.asarray(wk, dtype=np.float32)
    wv = np.asarray(wv, dtype=np.float32)
    wo = np.asarray(wo, dtype=np.float32)
    bf16 = ml_dtypes.bfloat16

    # RoPE tables (fp32, matching the reference's fp32 cos/sin); ss carries
    # the rotation signs for the half-swapped operand
    f = np.arange(0, DK, 2, dtype=np.float32) / DK          # 2f/d
    inv_freq = (ROPE_THETA ** (-f)).astype(np.float32)      # [64]
    ang = np.arange(S, dtype=np.float32)[:, None] * inv_freq[None, :]
    cos_t = np.cos(ang).T.astype(np.float32)                # [64, S]
    sin_t = np.sin(ang).T.astype(np.float32)
    cc = np.ascontiguousarray(np.vstack([cos_t, cos_t])).astype(bf16)  # [128,S]
    ss = np.ascontiguousarray(np.vstack([-sin_t, sin_t])).astype(bf16)

    tri = np.tril(np.ones((128, 128), dtype=np.float32)).T  # tri[j,i]=1 if j<=i
    tri = np.ascontiguousarray(tri).astype(bf16)

    deint = np.concatenate([np.arange(0, DK, 2), np.arange(1, DK, 2)])
    in_maps = []
    for c in range(NCORES):
        bi, g = divmod(c, 2)
        heads = [g * HPC + h for h in range(HPC)]
        qk_rows = np.concatenate([hg * DK + deint for hg in heads])
        v_rows = np.arange(g * DLOC, (g + 1) * DLOC)

        xt_t = _tile2(x[bi].T.astype(bf16), 128, 512)            # [DT,4,128,512]
        # w*_prep [d, e_loc] -> [DT, HPC, 128, DK] -> [HPC, DT, 128, DK]
        wq_t = _tile2(wq[qk_rows, :].T.astype(bf16), 128, DK).transpose(1, 0, 2, 3)
        wk_t = _tile2(wk[qk_rows, :].T.astype(bf16), 128, DK).transpose(1, 0, 2, 3)
        # wv pre-tiled g-major: [2, DT, 128, 512]
        wv_t = _tile2(wv[v_rows, :].T.astype(bf16), 128, 512).transpose(1, 0, 2, 3)
        # wo packed to land in the wv-shaped SBUF slot: [j=2*ndt+half,
        # p, c=ftl*128+f] holds wo.T[v_rows][ndt*128+p, (half*8+ftl)*128+f]
        wo_loc = wo.T[v_rows, :].astype(bf16)                     # [1024, 2048]
        tmp = wo_loc.reshape(NDT, 128, 2, 8, 128)                 # ndt,p,half,ftl,f
        wo_t = np.ascontiguousarray(
            tmp.transpose(0, 2, 1, 3, 4).reshape(DT, 128, DLOC))
        in_maps.append({
            "xt": np.ascontiguousarray(xt_t),
            "wq": np.ascontiguousarray(wq_t),
            "wk": np.ascontiguousarray(wk_t),
            "wv": np.ascontiguousarray(wv_t),
            "wo": wo_t,
            "cct": cc, "sst": ss,
            "tri": tri,
        })
    return in_maps


def assemble(results):
    out = np.empty((B, S, D), dtype=np.float32)
    for bi in range(B):
        oT = (results[2 * bi]["out"].astype(np.float32)
              + results[2 * bi + 1]["out"].astype(np.float32))
        # oT: [DT, IB, 128, 512] -> out^T [f, s]; out[b] = out^T.T
        oT = oT.transpose(0, 2, 1, 3).reshape(D, S)
        out[bi] = oT.T
    return out


def kernel(**inputs):
    nc = build_program()
    in_maps = prepare_in_maps(inputs["x"], inputs["wq"], inputs["wk"],
                              inputs["wv"], inputs["wo"])
    res = bass_utils.run_bass_kernel_spmd(nc, in_maps,
                                          core_ids=list(range(NCORES)))
    return assemble(res.results)
